# revision 8
# baseline (speedup 1.0000x reference)
"""KVGather (soft weights) Trainium2 Bass kernel.

out[b, i, k, w, c] = r_weight[b, i, k] * kv[b, r_idx[b, i, k], w, c]

Shapes (full): r_idx/r_weight (32, 49, 4), kv (32, 49, 64, 256),
out (32, 49, 4, 64, 256) f32.

Strategy: data-parallel over batch n=32 across 8 NeuronCores (4 samples
per core). Per sample, the 3.2 MB kv slab table is DMA'd into SBUF once
with layout [128 partitions, 49*128] (slab j at columns j*128, flat
(w,c) index = p*128 + f). Each of the 196 output slabs is produced by a
single DVE tensor_scalar multiply (f32 single-src -> 2x_2P perf mode)
reading the slab at a register-dynamic column offset (offset loaded
from an int32 offset table with values pre-scaled to idx*128) and
scaled by the per-partition-broadcast weight. Output chunks of 49 slabs
are DMA'd straight to DRAM in the exact output layout (512B contiguous
runs per partition). All DMA via HWDGE; loads are issued from nc.sync
(SP) and stores from nc.scalar (ACT) so they sit on independent FIFOs.
"""

import numpy as np

import concourse.bacc as bacc
import concourse.bass as bass
import concourse.mybir as mybir
import concourse.tile as tile
from concourse.bass_utils import run_bass_kernel_spmd

# Problem constants (hardcoded per harness contract).
N, P2, TOPK, W2, C = 32, 49, 4, 64, 256
NCORES = 8
NL = N // NCORES           # samples per core = 4
SLAB = W2 * C              # 16384 elements per gathered slab
IK = P2 * TOPK             # 196 output slabs per sample
PART = 128
FREE = SLAB // PART        # 128 columns per slab in SBUF layout
KV_COLS = P2 * FREE        # 6272
CHUNK = 49                 # output slabs per store chunk
NCHUNK = IK // CHUNK       # 4

_CACHE = {}


def build_bass():
    nc = bacc.Bacc("TRN2", target_bir_lowering=False)
    kv = nc.dram_tensor(
        "kv", [NL * P2, SLAB], mybir.dt.float32, kind="ExternalInput"
    )
    offs = nc.dram_tensor(
        "offs", [1, NL * IK], mybir.dt.int32, kind="ExternalInput"
    )
    wts = nc.dram_tensor(
        "wts", [PART, NL * IK], mybir.dt.float32, kind="ExternalInput"
    )
    out = nc.dram_tensor(
        "out", [NL * IK, SLAB], mybir.dt.float32, kind="ExternalOutput"
    )

    with tile.TileContext(nc) as tc:
        with (
            tc.tile_pool(name="misc", bufs=1) as misc,
            tc.tile_pool(name="kvp", bufs=2) as kvp,
            tc.tile_pool(name="outp", bufs=3) as outp,
        ):
            offs_t = misc.tile([1, NL * IK], mybir.dt.int32)
            wts_t = misc.tile([PART, NL * IK], mybir.dt.float32)
            nc.sync.dma_start(offs_t[:], offs[:])
            nc.sync.dma_start(wts_t[:], wts[:])



            for b in range(NL):
                kv_t = kvp.tile([PART, KV_COLS], mybir.dt.float32, tag="kv")
                nc.sync.dma_start(
                    kv_t[:].rearrange("p (j f) -> p j f", j=P2),
                    kv[b * P2 : (b + 1) * P2, :].rearrange(
                        "j (p f) -> p j f", p=PART
                    ),
                )
                for ch in range(NCHUNK):
                    out_t = outp.tile(
                        [PART, CHUNK * FREE], mybir.dt.float32, tag="out"
                    )
                    for s in range(CHUNK):
                        col = b * IK + ch * CHUNK + s
                        off = nc.values_load(
                            offs_t[0:1, col : col + 1],
                            engines=[mybir.EngineType.DVE],
                            min_val=0,
                            max_val=(P2 - 1) * FREE,
                            skip_runtime_bounds_check=True,
                        )
                        nc.vector.tensor_scalar_mul(
                            out_t[:, s * FREE : (s + 1) * FREE],
                            kv_t[:, bass.ds(off, FREE)],
                            wts_t[:, col : col + 1],
                        )
                    row0 = b * IK + ch * CHUNK
                    nc.scalar.dma_start(
                        out[row0 : row0 + CHUNK, :].rearrange(
                            "g (p f) -> p g f", p=PART
                        ),
                        out_t[:].rearrange("p (g f) -> p g f", g=CHUNK),
                    )
    nc.compile()
    return nc


def _get_nc():
    if "nc" not in _CACHE:
        _CACHE["nc"] = build_bass()
    return _CACHE["nc"]


def _make_in_maps(r_idx, r_weight, kv):
    in_maps = []
    for c in range(NCORES):
        lo, hi = c * NL, (c + 1) * NL
        kv_c = np.ascontiguousarray(
            kv[lo:hi].reshape(NL * P2, SLAB), dtype=np.float32
        )
        offs_c = np.ascontiguousarray(
            (r_idx[lo:hi].astype(np.int32) * FREE).reshape(1, NL * IK)
        )
        wts_c = np.ascontiguousarray(
            np.broadcast_to(
                r_weight[lo:hi].reshape(1, NL * IK).astype(np.float32),
                (PART, NL * IK),
            )
        )
        in_maps.append({"kv": kv_c, "offs": offs_c, "wts": wts_c})
    return in_maps


def kernel(r_idx, r_weight, kv):
    r_idx = np.asarray(r_idx)
    r_weight = np.asarray(r_weight)
    kv = np.asarray(kv)
    nc = _get_nc()
    in_maps = _make_in_maps(r_idx, r_weight, kv)
    res = run_bass_kernel_spmd(nc, in_maps, core_ids=list(range(NCORES)))
    outs = [
        res.results[c]["out"].reshape(NL, P2, TOPK, W2, C)
        for c in range(NCORES)
    ]
    return np.concatenate(outs, axis=0)
